# revision 8
# baseline (speedup 1.0000x reference)
"""FramePrimerDecoder Bass kernel for 8 trn2 NeuronCores.

Sharding: core c handles batch b = c//2 and band-pair bp = c%2 (bands
[2*bp, 2*bp+1]).  Bands are independent in attention; the only cross-band
mixing (o_proj's contraction over all 4 bands) is handled by a pairwise
AllGather of the per-band attention outputs (1 MB bf16), after which both
cores of a pair run the full o_proj / FFN redundantly and carry identical
residual state.

Everything on-chip lives transposed ([feature, w] with features on
partitions): the 1x1-conv input projection is then a pure elementwise
channel reduction, the depthwise conv-3 runs along the free axis, and the
qk/bias/av matmuls all consume natural layouts.  qk outputs are produced
transposed ([key, query]) and un-transposed on the host during unshard.
"""

import numpy as np
import ml_dtypes

B, CH, BINS, W = 4, 8, 1024, 1024
NB, D, FF = 4, 256, 2048
NBL = 2          # bands per core
EPS = 1e-5
RG = [[0, 1], [2, 3], [4, 5], [6, 7]]
SCALE = 1.0 / 32.0   # 1/sqrt(BINS)

_CACHE = {}


def _build():
    from contextlib import ExitStack
    import concourse.bacc as bacc
    import concourse.mybir as mybir
    import concourse.tile as tile

    f32 = mybir.dt.float32
    f32r = mybir.dt.float32r
    bf16 = mybir.dt.bfloat16
    MUL = mybir.AluOpType.mult
    ADD = mybir.AluOpType.add
    MAX = mybir.AluOpType.max
    AF = mybir.ActivationFunctionType

    nc = bacc.Bacc("TRN2", target_bir_lowering=False, debug=False,
                   num_devices=8)

    # ---------------- I/O ----------------
    xb = nc.dram_tensor("xb", [CH, BINS, W], f32, kind="ExternalInput")
    memb = nc.dram_tensor("memb", [CH, BINS, W], f32, kind="ExternalInput")
    prevT = [nc.dram_tensor(f"prevT{a}", [NBL, W, W], f32, kind="ExternalInput")
             for a in (1, 2)]
    qwT = [nc.dram_tensor(f"qwT{a}", [BINS, NBL * D], bf16, kind="ExternalInput") for a in (1, 2)]
    kwT = [nc.dram_tensor(f"kwT{a}", [BINS, NBL * D], bf16, kind="ExternalInput") for a in (1, 2)]
    vwT = [nc.dram_tensor(f"vwT{a}", [BINS, NBL * D], bf16, kind="ExternalInput") for a in (1, 2)]
    owT = [nc.dram_tensor(f"owT{a}", [BINS, BINS], bf16, kind="ExternalInput") for a in (1, 2)]
    pwd = [nc.dram_tensor(f"pw{a}", [D, W], bf16, kind="ExternalInput") for a in (1, 2)]
    qcd = [nc.dram_tensor(f"qc{a}", [128, 12], f32, kind="ExternalInput") for a in (1, 2)]
    kcd = [nc.dram_tensor(f"kc{a}", [128, 12], f32, kind="ExternalInput") for a in (1, 2)]
    vcd = [nc.dram_tensor(f"vc{a}", [128, 12], f32, kind="ExternalInput") for a in (1, 2)]
    l1wTd = nc.dram_tensor("l1wT", [BINS, FF], bf16, kind="ExternalInput")
    l2wTd = nc.dram_tensor("l2wT", [FF, BINS], bf16, kind="ExternalInput")
    gbd = nc.dram_tensor("gb", [128, 48], f32, kind="ExternalInput")  # g1,b1,g2,b2,g3,b3 each [128,8]
    cwd = nc.dram_tensor("cw", [128, 20], f32, kind="ExternalInput")  # ipw8, mpw8, ipb1, mpb1, eye? no: ip/mp only
    identd = nc.dram_tensor("ident", [128, 128], bf16, kind="ExternalInput")
    onesbd = nc.dram_tensor("onesb", [128, 128], bf16, kind="ExternalInput")
    onesfd = nc.dram_tensor("onesf", [128, 128], f32r, kind="ExternalInput")

    qkTo = [nc.dram_tensor(f"qk{a}T", [NBL, W, W], f32, kind="ExternalOutput")
            for a in (1, 2)]
    outTo = nc.dram_tensor("outT", [BINS, W], f32, kind="ExternalOutput")

    with tile.TileContext(nc, num_cores=8) as tc, ExitStack() as top:
        # ------------- persistent pools -------------
        p_xs = top.enter_context(tc.tile_pool(name="xs", bufs=1))
        p_ln = top.enter_context(tc.tile_pool(name="ln", bufs=1))
        p_const = top.enter_context(tc.tile_pool(name="const", bufs=1))
        p_lnstat = top.enter_context(tc.tile_pool(name="lnstat", bufs=2))
        p_sq = top.enter_context(tc.tile_pool(name="sq", bufs=3))
        p_xin = top.enter_context(tc.tile_pool(name="xin", bufs=2))
        p_wk = top.enter_context(tc.tile_pool(name="wk", bufs=8))
        p_ps = top.enter_context(tc.tile_pool(name="ps", bufs=7, space="PSUM"))
        p_dram = top.enter_context(tc.tile_pool(name="ccd", bufs=1, space="DRAM"))

        xsT = p_xs.tile([128, 8 * W], f32)         # residual stream, transposed
        lnT = p_ln.tile([128, 8 * W], bf16)        # LN output (reused 3x)

        identt = p_const.tile([128, 128], bf16, tag="c1")
        onesb = p_const.tile([128, 128], bf16, tag="c2")
        onesf = p_const.tile([128, 128], f32r, tag="c3")
        gbt = p_const.tile([128, 48], f32, tag="c4")
        cwt = p_const.tile([128, 20], f32, tag="c5")
        qct = [p_const.tile([128, 12], f32, tag=f"c6{a}", name=f"qct{a}") for a in (0, 1)]
        kct = [p_const.tile([128, 12], f32, tag=f"c7{a}", name=f"kct{a}") for a in (0, 1)]
        vct = [p_const.tile([128, 12], f32, tag=f"c8{a}", name=f"vct{a}") for a in (0, 1)]
        nc.sync.dma_start(out=identt[:], in_=identd[:])
        nc.sync.dma_start(out=onesb[:], in_=onesbd[:])
        nc.sync.dma_start(out=onesf[:], in_=onesfd[:])
        nc.sync.dma_start(out=gbt[:], in_=gbd[:])
        nc.sync.dma_start(out=cwt[:], in_=cwd[:])
        for a in (0, 1):
            nc.sync.dma_start(out=qct[a][:], in_=qcd[a][:])
            nc.sync.dma_start(out=kct[a][:], in_=kcd[a][:])
            nc.sync.dma_start(out=vct[a][:], in_=vcd[a][:])

        IPW, MPW, IPB, MPB = cwt[:, 0:8], cwt[:, 8:16], cwt[:, 16:17], cwt[:, 17:18]

        def chanreduce_f32(src, out_tile, wv, bv):
            # out[bin, w] = sum_c wv[c] * src[c, bin, w] + bv ; out f32 in place
            for t in range(8):
                o = out_tile[:, t * W:(t + 1) * W]
                for c in range(CH):
                    xt = p_xin.tile([128, W], f32, tag="xin")
                    nc.sync.dma_start(out=xt[:], in_=src[c, t * 128:(t + 1) * 128, :])
                    if c == 0:
                        nc.vector.tensor_scalar(o, xt[:], wv[:, 0:1], None, MUL)
                    else:
                        nc.vector.scalar_tensor_tensor(o, xt[:], wv[:, c:c + 1], o, MUL, ADD)
                nc.vector.tensor_scalar(o, o, bv, None, ADD)

        def chanreduce_bf16(src, out_tile, wv, bv):
            # same but accumulate in f32 scratch, emit bf16
            for t in range(8):
                acc = p_sq.tile([128, W], f32, tag="sq")
                for c in range(CH):
                    xt = p_xin.tile([128, W], f32, tag="xin")
                    nc.sync.dma_start(out=xt[:], in_=src[c, t * 128:(t + 1) * 128, :])
                    if c == 0:
                        nc.vector.tensor_scalar(acc[:], xt[:], wv[:, 0:1], None, MUL)
                    else:
                        nc.vector.scalar_tensor_tensor(acc[:], xt[:], wv[:, c:c + 1], acc[:], MUL, ADD)
                nc.vector.tensor_scalar(out_tile[:, t * W:(t + 1) * W], acc[:], bv, None, ADD)

        def layernorm(lni):
            # stats over partition(bin) axis of xsT via ones-matmuls (f32r),
            # broadcast [128, W] mean/rstd, apply -> lnT bf16
            g = gbt[:, lni * 16:lni * 16 + 8]
            b = gbt[:, lni * 16 + 8:lni * 16 + 16]
            s1 = [p_ps.tile([128, 512], f32, tag="ps", name=f"s1_{i}") for i in range(2)]
            s2 = [p_ps.tile([128, 512], f32, tag="ps", name=f"s2_{i}") for i in range(2)]
            lhs1 = onesf[:]
            for t in range(8):
                xr = p_sq.tile([128, W], f32r, tag="sq", name=f"xr{t}")
                nc.scalar.copy(xr[:], xsT[:, t * W:(t + 1) * W])
                sqt = p_sq.tile([128, W], f32r, tag="sq")
                nc.scalar.activation(sqt[:], xsT[:, t * W:(t + 1) * W], AF.Square)
                for ih in (0, 1):
                    nc.tensor.matmul(s1[ih][:], lhs1, xr[:, ih * 512:ih * 512 + 512],
                                     start=(t == 0), stop=(t == 7))
                    nc.tensor.matmul(s2[ih][:], lhs1, sqt[:, ih * 512:ih * 512 + 512],
                                     start=(t == 0), stop=(t == 7))
            meanb = p_lnstat.tile([128, W], f32, tag="lnstat")
            rstdb = p_lnstat.tile([128, W], f32, tag="lnstat")
            for ih in (0, 1):
                sl = slice(ih * 512, ih * 512 + 512)
                nc.vector.tensor_scalar(meanb[:, sl], s1[ih][:], 1.0 / BINS, None, MUL)
                nc.vector.tensor_scalar(rstdb[:, sl], s2[ih][:], 1.0 / BINS, None, MUL)
            tmp = p_sq.tile([128, W], f32, tag="sq")
            nc.vector.tensor_mul(tmp[:], meanb[:], meanb[:])
            nc.vector.tensor_sub(rstdb[:], rstdb[:], tmp[:])
            nc.scalar.activation(tmp[:], rstdb[:], AF.Sqrt, bias=cwt[:, 18:19])
            nc.vector.reciprocal(rstdb[:], tmp[:])
            for t in range(8):
                o = slice(t * W, (t + 1) * W)
                tm = p_sq.tile([128, W], f32, tag="sq")
                nc.vector.tensor_sub(tm[:], xsT[:, o], meanb[:])
                nc.vector.tensor_mul(tm[:], tm[:], rstdb[:])
                nc.vector.tensor_scalar(lnT[:, o], tm[:], g[:, t:t + 1], b[:, t:t + 1], MUL, ADD)

        def proj_conv(wdram, rhs_tile, conv_t, out_tile, p_stg):
            # out_tile[:, r*W:(r+1)*W] = dwconv3(wdram.T @ rhs), rows r*128..
            wt = []
            for gk in range(8):
                t = p_wk.tile([128, 512], bf16, tag="wk")
                nc.sync.dma_start(out=t[:], in_=wdram[gk * 128:(gk + 1) * 128, :])
                wt.append(t)
            for r in range(4):
                stg = p_stg.tile([128, 1026], bf16, tag="stg")
                nc.vector.memset(stg[:, 0:1], 0.0)
                nc.vector.memset(stg[:, 1025:1026], 0.0)
                for ih in (0, 1):
                    pt = p_ps.tile([128, 512], f32, tag="ps")
                    for gk in range(8):
                        nc.tensor.matmul(pt[:], wt[gk][:, r * 128:(r + 1) * 128],
                                         rhs_tile[:, gk * W + ih * 512: gk * W + ih * 512 + 512],
                                         start=(gk == 0), stop=(gk == 7))
                    nc.scalar.copy(stg[:, 1 + ih * 512:1 + ih * 512 + 512], pt[:])
                o = out_tile[:, r * W:(r + 1) * W]
                nc.vector.tensor_scalar(o, stg[:, 0:W], conv_t[:, r * 3:r * 3 + 1], None, MUL)
                nc.vector.scalar_tensor_tensor(o, stg[:, 1:1 + W], conv_t[:, r * 3 + 1:r * 3 + 2], o, MUL, ADD)
                nc.vector.scalar_tensor_tensor(o, stg[:, 2:2 + W], conv_t[:, r * 3 + 2:r * 3 + 3], o, MUL, ADD)

        # =====================================================
        with ExitStack() as attn_scope:
            p_ms = attn_scope.enter_context(tc.tile_pool(name="ms", bufs=1))
            p_q = attn_scope.enter_context(tc.tile_pool(name="q", bufs=1))
            p_k = attn_scope.enter_context(tc.tile_pool(name="k", bufs=1))
            p_v = attn_scope.enter_context(tc.tile_pool(name="v", bufs=1))
            p_stg = attn_scope.enter_context(tc.tile_pool(name="stg", bufs=2))
            p_vnat = attn_scope.enter_context(tc.tile_pool(name="vnat", bufs=1))
            p_aT = attn_scope.enter_context(tc.tile_pool(name="aT", bufs=1))
            p_qks = attn_scope.enter_context(tc.tile_pool(name="qks", bufs=2))
            p_prev = attn_scope.enter_context(tc.tile_pool(name="prev", bufs=2))
            p_recip = attn_scope.enter_context(tc.tile_pool(name="recip", bufs=1))
            p_avloc = attn_scope.enter_context(tc.tile_pool(name="avloc", bufs=1))
            p_avfull = attn_scope.enter_context(tc.tile_pool(name="avfull", bufs=1))
            p_pw = attn_scope.enter_context(tc.tile_pool(name="pw", bufs=1))
            p_ow = attn_scope.enter_context(tc.tile_pool(name="ow", bufs=8))

            msT = p_ms.tile([128, 8 * W], bf16)
            qT = p_q.tile([128, 4 * W], bf16)
            kT = p_k.tile([128, 4 * W], bf16)
            vT = p_v.tile([128, 4 * W], bf16)
            aT = p_aT.tile([128, 8 * W], bf16)
            recipS = p_recip.tile([128, W], f32)

            av_in = [p_dram.tile([NBL * D, W], bf16, tag=f"avin{a}", name=f"av_in{a}") for a in (0, 1)]
            av_out = [p_dram.tile([BINS, W], bf16, tag=f"avout{a}", name=f"av_out{a}") for a in (0, 1)]

            # ---- stage 0: xs, LN1, qkv1 ----
            chanreduce_f32(xb, xsT, IPW, IPB)
            def load_pw(a):
                t = p_pw.tile([128, 2 * W], bf16, tag="pw")
                for dk in (0, 1):
                    nc.sync.dma_start(out=t[:, dk * W:(dk + 1) * W],
                                      in_=pwd[a][dk * 128:(dk + 1) * 128, :])
                return t
            pwt = [None, None]
            pwt[0] = load_pw(0)
            layernorm(0)
            proj_conv(qwT[0], lnT, qct[0], qT, p_stg)
            proj_conv(kwT[0], lnT, kct[0], kT, p_stg)
            proj_conv(vwT[0], lnT, vct[0], vT, p_stg)

            def band_attn(a, n):
                # vnat: [j, d] blocks for this band
                vn = p_vnat.tile([128, 2048], bf16, tag="vnat")
                for jt in range(8):
                    for dk in (0, 1):
                        pst = p_ps.tile([128, 128], bf16, tag="ps")
                        nc.tensor.transpose(pst[:], vT[:, (2 * n + dk) * W + jt * 128:(2 * n + dk) * W + jt * 128 + 128],
                                            identt[:])
                        nc.scalar.copy(vn[:, jt * 256 + dk * 128: jt * 256 + dk * 128 + 128], pst[:])
                # qkT tiles: [j, i], + bias, + prev; exp -> aT
                for jt in range(8):
                    for ih in (0, 1):
                        pvh = p_prev.tile([128, 512], f32, tag="prev")
                        nc.sync.dma_start(out=pvh[:], in_=prevT[a][n, jt * 128:(jt + 1) * 128, ih * 512:ih * 512 + 512])
                        pq = p_ps.tile([128, 512], f32, tag="ps")
                        for dk in (0, 1):
                            nc.tensor.matmul(pq[:], kT[:, (2 * n + dk) * W + jt * 128:(2 * n + dk) * W + jt * 128 + 128],
                                             qT[:, (2 * n + dk) * W + ih * 512:(2 * n + dk) * W + ih * 512 + 512],
                                             start=(dk == 0), stop=False)
                        for dk in (0, 1):
                            nc.tensor.matmul(pq[:], qT[:, (2 * n + dk) * W + jt * 128:(2 * n + dk) * W + jt * 128 + 128],
                                             pwt[a][:, dk * W + ih * 512:dk * W + ih * 512 + 512],
                                             start=False, stop=(dk == 1))
                        qks = p_qks.tile([128, 512], f32, tag="qks")
                        nc.vector.scalar_tensor_tensor(qks[:], pq[:], SCALE, pvh[:], MUL, ADD)
                        nc.sync.dma_start(out=qkTo[a][n, jt * 128:(jt + 1) * 128, ih * 512:ih * 512 + 512], in_=qks[:])
                        nc.scalar.activation(aT[:, jt * W + ih * 512: jt * W + ih * 512 + 512], qks[:], AF.Exp)
                # denominators (broadcast across partitions via all-ones lhsT)
                for ih in (0, 1):
                    dps = p_ps.tile([128, 512], f32, tag="ps")
                    for jt in range(8):
                        nc.tensor.matmul(dps[:], onesb[:], aT[:, jt * W + ih * 512: jt * W + ih * 512 + 512],
                                         start=(jt == 0), stop=(jt == 7))
                    nc.vector.reciprocal(recipS[:, ih * 512:ih * 512 + 512], dps[:])
                # av, normalized -> avloc bf16
                avl = p_avloc.tile([128, 2048], bf16, tag="avloc")
                for dm in (0, 1):
                    for ih in (0, 1):
                        pa = p_ps.tile([128, 512], f32, tag="ps")
                        for jt in range(8):
                            nc.tensor.matmul(pa[:], vn[:, jt * 256 + dm * 128: jt * 256 + dm * 128 + 128],
                                             aT[:, jt * W + ih * 512: jt * W + ih * 512 + 512],
                                             start=(jt == 0), stop=(jt == 7))
                        nc.vector.scalar_tensor_tensor(avl[:, dm * W + ih * 512: dm * W + ih * 512 + 512],
                                                       pa[:], 1.0, recipS[:, ih * 512:ih * 512 + 512], MUL, MUL)
                for dm in (0, 1):
                    nc.sync.dma_start(out=av_in[a][n * 256 + dm * 128: n * 256 + (dm + 1) * 128, :],
                                      in_=avl[:, dm * W:(dm + 1) * W])

            def oproj(a):
                avfull = p_avfull.tile([128, 8 * W], bf16, tag="avfull")
                for gk in range(8):
                    nc.sync.dma_start(out=avfull[:, gk * W:(gk + 1) * W],
                                      in_=av_out[a][gk * 128:(gk + 1) * 128, :])
                for h in (0, 1):
                    owh = []
                    for gk in range(8):
                        t = p_ow.tile([128, 512], bf16, tag="ow")
                        nc.sync.dma_start(out=t[:], in_=owT[a][gk * 128:(gk + 1) * 128, h * 512:h * 512 + 512])
                        owh.append(t)
                    for cl in range(4):
                        ct = 4 * h + cl
                        for ih in (0, 1):
                            po = p_ps.tile([128, 512], f32, tag="ps")
                            for gk in range(8):
                                nc.tensor.matmul(po[:], owh[gk][:, cl * 128:(cl + 1) * 128],
                                                 avfull[:, gk * W + ih * 512: gk * W + ih * 512 + 512],
                                                 start=(gk == 0), stop=(gk == 7))
                            xsl = xsT[:, ct * W + ih * 512: ct * W + ih * 512 + 512]
                            nc.vector.scalar_tensor_tensor(xsl, po[:], 1.0, xsl, MUL, ADD)

            # ---- attn1 ----
            for n in (0, 1):
                band_attn(0, n)
            nc.gpsimd.collective_compute(
                "AllGather", mybir.AluOpType.bypass, replica_groups=RG,
                ins=[av_in[0].opt()], outs=[av_out[0].opt()])
            # overlap AG latency: ms chanreduce + attn2 k/v projections
            chanreduce_bf16(memb, msT, MPW, MPB)
            proj_conv(vwT[1], msT, vct[1], vT, p_stg)
            proj_conv(kwT[1], msT, kct[1], kT, p_stg)
            oproj(0)
            # ---- attn2 ----
            layernorm(1)
            proj_conv(qwT[1], lnT, qct[1], qT, p_stg)
            pwt[1] = load_pw(1)
            for n in (0, 1):
                band_attn(1, n)
            nc.gpsimd.collective_compute(
                "AllGather", mybir.AluOpType.bypass, replica_groups=RG,
                ins=[av_in[1].opt()], outs=[av_out[1].opt()])
            oproj(1)

        # ---- FFN ----
        layernorm(2)
        with ExitStack() as ffn_scope:
            p_ff = ffn_scope.enter_context(tc.tile_pool(name="ff", bufs=1))
            p_rl = ffn_scope.enter_context(tc.tile_pool(name="rl", bufs=3))
            ffT = p_ff.tile([128, 16 * W], bf16)
            with tc.tile_pool(name="l1w", bufs=8) as p_l1:
                l1t = []
                for gk in range(8):
                    t = p_l1.tile([128, FF], bf16, tag="l1w")
                    nc.sync.dma_start(out=t[:], in_=l1wTd[gk * 128:(gk + 1) * 128, :])
                    l1t.append(t)
                for ft in range(16):
                    for ih in (0, 1):
                        pf = p_ps.tile([128, 512], f32, tag="ps")
                        for gk in range(8):
                            nc.tensor.matmul(pf[:], l1t[gk][:, ft * 128:(ft + 1) * 128],
                                             lnT[:, gk * W + ih * 512: gk * W + ih * 512 + 512],
                                             start=(gk == 0), stop=(gk == 7))
                        rl = p_rl.tile([128, 512], bf16, tag="rl")
                        nc.scalar.activation(rl[:], pf[:], AF.Relu)
                        nc.vector.tensor_mul(
                            ffT[:, ft * W + ih * 512: ft * W + ih * 512 + 512],
                            rl[:], rl[:])
            with tc.tile_pool(name="l2w", bufs=16) as p_l2:
                l2t = []
                for fk in range(16):
                    t = p_l2.tile([128, W], bf16, tag="l2w")
                    nc.sync.dma_start(out=t[:], in_=l2wTd[fk * 128:(fk + 1) * 128, :])
                    l2t.append(t)
                for ct in range(8):
                    for ih in (0, 1):
                        po = p_ps.tile([128, 512], f32, tag="ps")
                        for fk in range(16):
                            nc.tensor.matmul(po[:], l2t[fk][:, ct * 128:(ct + 1) * 128],
                                             ffT[:, fk * W + ih * 512: fk * W + ih * 512 + 512],
                                             start=(fk == 0), stop=(fk == 15))
                        xsl = xsT[:, ct * W + ih * 512: ct * W + ih * 512 + 512]
                        nc.vector.scalar_tensor_tensor(xsl, po[:], 1.0, xsl, MUL, ADD)

        for t in range(8):
            nc.sync.dma_start(out=outTo[t * 128:(t + 1) * 128, :], in_=xsT[:, t * W:(t + 1) * W])

    nc.compile()
    return nc


def _bf(x):
    return np.ascontiguousarray(x.astype(ml_dtypes.bfloat16))


def _prep_maps(inputs):
    """Build the 8 per-core input maps from the full problem inputs."""
    f = {k: np.asarray(v) for k, v in inputs.items()}
    gb = np.zeros((128, 48), np.float32)
    for i, nm in enumerate(["ln1_g", "ln1_b", "ln2_g", "ln2_b", "ln3_g", "ln3_b"]):
        # column t of block = bins tile t -> [128, 8]
        blk = f[nm].reshape(8, 128).T
        gb[:, (i // 2) * 16 + (i % 2) * 8:(i // 2) * 16 + (i % 2) * 8 + 8] = blk
    cw = np.zeros((128, 20), np.float32)
    cw[:, 0:8] = np.broadcast_to(f["ip_w"], (128, 8))
    cw[:, 8:16] = np.broadcast_to(f["mp_w"], (128, 8))
    cw[:, 16:17] = float(f["ip_b"][0])
    cw[:, 17:18] = float(f["mp_b"][0])
    cw[:, 18:19] = EPS
    ident = np.eye(128, dtype=np.float32)
    common = {
        "l1wT": _bf(f["l1_w"].T), "l2wT": _bf(f["l2_w"].T),
        "gb": gb, "cw": cw,
        "ident": _bf(ident), "onesb": _bf(np.ones((128, 128), np.float32)),
        "onesf": np.ones((128, 128), np.float32),
    }
    for ai, a in enumerate(("1", "2")):
        common[f"pw{a}"] = _bf(f[f"a{a}_pw"])
    maps = []
    for core in range(8):
        b, bp = core // 2, core % 2
        bands = slice(2 * bp, 2 * bp + 2)
        chans = slice(2 * bp * D, (2 * bp + 2) * D)
        m = dict(common)
        m["xb"] = np.ascontiguousarray(f["x"][b])
        m["memb"] = np.ascontiguousarray(f["mem"][b])
        for a in ("1", "2"):
            m[f"prevT{a}"] = np.ascontiguousarray(
                f[f"prev_qk{a}"][b, bands].transpose(0, 2, 1))
            m[f"qwT{a}"] = _bf(f[f"a{a}_qw"].T[:, chans])
            m[f"kwT{a}"] = _bf(f[f"a{a}_kw"].T[:, chans])
            m[f"vwT{a}"] = _bf(f[f"a{a}_vw"].T[:, chans])
            m[f"owT{a}"] = _bf(f[f"a{a}_ow"].T)
            for w in ("q", "k", "v"):
                # conv weights for my 512 channels -> [128, 12] (4 ptiles x 3 taps)
                c = f[f"a{a}_{w}c"][chans.start:chans.stop, 0, :]   # [512, 3]
                m[f"{w}c{a}"] = np.ascontiguousarray(
                    c.reshape(4, 128, 3).transpose(1, 0, 2).reshape(128, 12))
        maps.append(m)
    return maps


def kernel(**inputs):
    from concourse.bass_utils import run_bass_kernel_spmd

    if "nc" not in _CACHE:
        _CACHE["nc"] = _build()
    nc = _CACHE["nc"]
    maps = _prep_maps(inputs)
    r = run_bass_kernel_spmd(nc, maps, core_ids=list(range(8)))
    _CACHE["last_result"] = r
    res = r.results

    out = np.empty((B, 1, BINS, W), np.float32)
    qk1 = np.empty((B, NB, W, W), np.float32)
    qk2 = np.empty((B, NB, W, W), np.float32)
    for core in range(8):
        b, bp = core // 2, core % 2
        r = res[core]
        if bp == 0:
            out[b, 0] = r["outT"]
        for n in range(NBL):
            qk1[b, 2 * bp + n] = r["qk1T"][n].T
            qk2[b, 2 * bp + n] = r["qk2T"][n].T
    return (out, qk1, qk2)
